# revision 25
# baseline (speedup 1.0000x reference)
"""Trainium2 Bass kernel for 2-layer LSTM (H=64) + linear head. bf16 v3.

Math (PyTorch gate order i,f,g,o):
  per layer: z = W_hh @ h + W_ih @ x + b;  i,f,o = sigmoid; g = tanh
             c = f*c + i*g ; h = o*tanh(c)
  out = h2[:, -1] @ Wlin.T + blin

Per-chunk activation funcs (the two gate chunks use separate ACT instrs,
which also lets ACT-A start while chunk1's matmuls still run):
  chunk0 [i;f] -> Sigmoid (true si, sf)
  chunk1 [g;o] -> Tanh; o rows pre-scaled 0.5 so yo = 2*sigmoid(z_o)-1
Elementwise per step (TT ops get bf16 2x mode; the one STT is 1x):
  u  = si * tg                # TT   = i*g
  v  = sf * C                 # TT   = f*c
  C' = u + v                  # TT   = c'
  TC = tanh(C')               # ACT
  h' = (yo + 1) * TC          # STT  = 2h; consumer weight cols carry 0.5

Kernel layout (per core, batch B_L=256 split into 2 streams of BS=128):
  - State transposed [H, batch], layers fused along free dim (cols 0:BS =
    layer1 at t=k, cols BS:2BS = layer2 at t=k-1; layer2 lags one step).
  - rr tile [71, 2BS] bf16: rows 0:64 h, row 64 ones, rows 65:71 x_t^T.
  - psG [128, 4BS] fp32 = one PSUM bank: cols 0:2BS = chunk [i;f],
    cols 2BS:4BS = chunk [g;o]. 6 matmuls per step fill it; ONE Sigmoid ACT
    reads all 512 cols (mid steps).
  - x passed host-transposed [I, T, B] so the per-step DMA is 6 contiguous
    256B runs instead of a 768-descriptor 2-byte gather.
"""

import numpy as np

H = 64
I = 6
O = 6
NCORES = 8


def _build(nc, tc, BL, BS, T, dt):
    import concourse.bass as bass
    from concourse import mybir

    f32 = mybir.dt.float32
    bf16 = mybir.dt.bfloat16
    AF = mybir.ActivationFunctionType
    OP = mybir.AluOpType
    nstreams = BL // BS

    xt_d = nc.dram_tensor("xt", [I, T, BL], bf16, kind="ExternalInput")
    w1_d = nc.dram_tensor("w1", [71, 256], bf16, kind="ExternalInput")
    w2a_d = nc.dram_tensor("w2a", [64, 256], bf16, kind="ExternalInput")
    w2b_d = nc.dram_tensor("w2b", [65, 256], bf16, kind="ExternalInput")
    wl_d = nc.dram_tensor("wl", [65, O], bf16, kind="ExternalInput")
    y_d = nc.dram_tensor("y", [BL, O], f32, kind="ExternalOutput")

    yT = y_d[:, :].rearrange("b o -> o b")                 # [O, BL]

    import contextlib
    ctx = contextlib.ExitStack()
    wp = ctx.enter_context(tc.tile_pool(name="w", bufs=1))
    rrp = ctx.enter_context(tc.tile_pool(name="rr", bufs=3))
    cp = ctx.enter_context(tc.tile_pool(name="c", bufs=2))
    sp = ctx.enter_context(tc.tile_pool(name="s", bufs=3))
    pp = ctx.enter_context(tc.tile_pool(name="ps", bufs=1, space="PSUM"))
    pfp = ctx.enter_context(tc.tile_pool(name="psf", bufs=1, space="PSUM"))

    # --- weights to SBUF ---
    w1c = []
    w2ac = []
    w2bc = []
    for c in range(2):
        t_ = wp.tile([71, 128], bf16, tag=f"w1c{c}")
        nc.sync.dma_start(out=t_, in_=w1_d[:, c * 128:(c + 1) * 128])
        w1c.append(t_)
        t_ = wp.tile([64, 128], bf16, tag=f"w2a{c}")
        nc.sync.dma_start(out=t_, in_=w2a_d[:, c * 128:(c + 1) * 128])
        w2ac.append(t_)
        t_ = wp.tile([65, 128], bf16, tag=f"w2b{c}")
        nc.sync.dma_start(out=t_, in_=w2b_d[:, c * 128:(c + 1) * 128])
        w2bc.append(t_)
    wl = wp.tile([65, O], bf16, tag="wl")
    nc.sync.dma_start(out=wl, in_=wl_d[:, :])

    A = slice(0, BS)          # layer-1 cols
    Bc = slice(BS, 2 * BS)    # layer-2 cols
    F = slice(0, 2 * BS)

    for s in range(nstreams):
        bs0 = s * BS

        # persistent ring tiles
        rr = [rrp.tile([71, 2 * BS], dt, tag=f"rr{s}", name=f"rr{s}_{j}")
              for j in range(3)]
        cst = [cp.tile([128, 2 * BS], dt, tag=f"c{s}", name=f"c{s}_{j}")
               for j in range(2)]
        for t_ in rr:
            nc.vector.memset(t_[0:64, :], 0.0)
            nc.vector.memset(t_[64:65, :], 1.0)
        for t_ in cst:
            nc.vector.memset(t_[64:128, :], 0.0)

        # x for tick 0
        nc.sync.dma_start(out=rr[0][65:71, 0:BS], in_=xt_d[:, 0, bs0:bs0 + BS])

        for k in range(T + 1):
            # logical-clock hint: phase-lock the two streams half a step
            # apart so the scheduler's queues interleave s0/s1 work at the
            # offset the hardware actually runs (keeps s1's gate-ACTs from
            # queueing ahead of s0's tanh(c))
            tc.tile_set_cur_wait(k * 0.0034 + s * 0.0017)
            do1 = k < T
            do2 = k > 0
            cols = F if (do1 and do2) else (A if do1 else Bc)
            rcur = rr[k % 3]
            rnxt = rr[(k + 1) % 3]
            ccur = cst[k % 2]
            cnxt = cst[(k + 1) % 2]

            psA = pp.tile([128, 2 * BS], f32, tag=f"pA{s}")
            psB = pp.tile([128, 2 * BS], f32, tag=f"pB{s}")
            # chunk0 ([i;f] rows -> psA) matmuls first so ACT-A can start
            # while chunk1's matmuls still run
            if do1:
                nc.tensor.matmul(psA[:, A], w1c[0], rcur[0:71, A],
                                 start=True, stop=True)
            if do2:
                nc.tensor.matmul(psA[:, Bc], w2ac[0], rcur[0:64, Bc],
                                 start=True, stop=False)
                nc.tensor.matmul(psA[:, Bc], w2bc[0], rcur[0:65, A],
                                 start=False, stop=True)
            if do1:
                nc.tensor.matmul(psB[:, A], w1c[1], rcur[0:71, A],
                                 start=True, stop=True)
            if do2:
                nc.tensor.matmul(psB[:, Bc], w2ac[1], rcur[0:64, Bc],
                                 start=True, stop=False)
                nc.tensor.matmul(psB[:, Bc], w2bc[1], rcur[0:65, A],
                                 start=False, stop=True)

            # gate activations, split per chunk (ACT-A overlaps chunk1's
            # matmuls): chunk0 [i;f] -> Sigmoid, chunk1 [g;o] -> Tanh with
            # o-rows pre-scaled 0.5 so yo = 2*sigmoid(z_o)-1.
            Y = sp.tile([128, 4 * BS], dt, tag=f"Y{s}")
            nc.scalar.activation(Y[:, cols], psA[:, cols], AF.Sigmoid)
            nc.scalar.activation(Y[:, 2 * BS + cols.start:2 * BS + cols.stop],
                                 psB[:, cols], AF.Tanh)

            Si = Y[0:64, 0:2 * BS]
            Sf = Y[64:128, 0:2 * BS]
            Tg = Y[0:64, 2 * BS:4 * BS]
            Yo = Y[64:128, 2 * BS:4 * BS]

            u = sp.tile([64, 2 * BS], dt, tag=f"u{s}")
            v = sp.tile([64, 2 * BS], dt, tag=f"v{s}")
            # v = sf * C   [= f*c]  (needs only ACT-A)
            nc.vector.tensor_tensor(
                out=v[:, cols], in0=Sf[:, cols], in1=ccur[64:128, cols],
                op=OP.mult)
            # u = si * tg  [= i*g]
            nc.vector.tensor_tensor(
                out=u[:, cols], in0=Si[:, cols], in1=Tg[:, cols], op=OP.mult)
            # C' = u + v
            nc.vector.tensor_tensor(
                out=cnxt[64:128, cols], in0=u[:, cols], in1=v[:, cols],
                op=OP.add)
            # TC = tanh(C')
            TC = sp.tile([128, 2 * BS], dt, tag=f"TC{s}")
            nc.scalar.activation(TC[64:128, cols], cnxt[64:128, cols], AF.Tanh)
            # h' = (yo + 1) * TC   [= 2h; the 2x is folded into consumer cols]
            nc.vector.scalar_tensor_tensor(
                out=rnxt[0:64, cols], in0=Yo[:, cols], scalar=1.0,
                in1=TC[64:128, cols], op0=OP.add, op1=OP.mult)

            if k + 1 < T:
                nc.sync.dma_start(out=rnxt[65:71, 0:BS],
                                  in_=xt_d[:, k + 1, bs0:bs0 + BS])

        # final linear: y = [Wlin | blin] @ [h2; 1]
        rfin = rr[(T + 1) % 3]
        psF = pfp.tile([O, BS], f32, tag=f"pF{s}")
        nc.tensor.matmul(psF[:, :], wl, rfin[0:65, Bc], start=True, stop=True)
        oF = sp.tile([O, BS], f32, tag=f"oF{s}")
        nc.vector.tensor_copy(oF[:, :], psF[:, :])
        nc.sync.dma_start(out=yT[:, bs0:bs0 + BS], in_=oF)

    ctx.close()


def build_nc(BL=256, BS=128, T=512, dtype="bfloat16"):
    import concourse.bacc as bacc
    import concourse.tile as tile
    from concourse import mybir

    dt = getattr(mybir.dt, dtype)
    nc = bacc.Bacc(None, target_bir_lowering=False)
    with tile.TileContext(nc) as tc:
        _build(nc, tc, BL, BS, T, dt)
    nc.compile()
    return nc


def prep_weights(Wih0, Whh0, bih0, bhh0, Wih1, Whh1, bih1, bhh1, Wlin, blin):
    """Host-side weight prep. Returns dict of bf16 DRAM tensors."""
    import ml_dtypes
    f = np.float32
    bf = ml_dtypes.bfloat16
    b0 = (bih0 + bhh0).astype(f)
    b1 = (bih1 + bhh1).astype(f)

    def oscale(M):  # scale o-gate rows (192:256) by 0.5
        M = M.copy()
        M[192:256] *= 0.5
        return M

    # h' = 2h convention: consumer cols of h get * 0.5
    w1 = oscale(np.concatenate([Whh0 * 0.5, b0[:, None], Wih0],
                               axis=1).astype(f))
    w2a = oscale((Whh1 * 0.5).astype(f))
    w2b = oscale(np.concatenate([Wih1 * 0.5, b1[:, None]], axis=1).astype(f))
    wlin_aug = np.concatenate([Wlin * 0.5, blin[:, None]], axis=1).astype(f)

    return {
        "w1": np.ascontiguousarray(w1.T).astype(bf),       # [71, 256]
        "w2a": np.ascontiguousarray(w2a.T).astype(bf),     # [64, 256]
        "w2b": np.ascontiguousarray(w2b.T).astype(bf),     # [65, 256]
        "wl": np.ascontiguousarray(wlin_aug.T).astype(bf), # [65, 6]
    }


_NC_CACHE = {}


def kernel(x, Wih0, Whh0, bih0, bhh0, Wih1, Whh1, bih1, bhh1, Wlin, blin,
           _trace=False):
    import ml_dtypes
    from concourse.bass_utils import run_bass_kernel_spmd

    x = np.asarray(x, dtype=np.float32)
    B, T, _ = x.shape
    BL = B // NCORES
    key = (BL, T)
    if key not in _NC_CACHE:
        _NC_CACHE[key] = build_nc(BL=BL, BS=BL // 2, T=T)
    nc = _NC_CACHE[key]

    # host transpose to [I, T, B] so per-step DMA slices are contiguous
    xt = np.ascontiguousarray(np.transpose(x, (2, 1, 0))).astype(
        ml_dtypes.bfloat16)

    w = prep_weights(np.asarray(Wih0), np.asarray(Whh0), np.asarray(bih0),
                     np.asarray(bhh0), np.asarray(Wih1), np.asarray(Whh1),
                     np.asarray(bih1), np.asarray(bhh1), np.asarray(Wlin),
                     np.asarray(blin))

    in_maps = []
    for c in range(NCORES):
        m = {"xt": np.ascontiguousarray(xt[:, :, c * BL:(c + 1) * BL])}
        m.update(w)
        in_maps.append(m)

    res = run_bass_kernel_spmd(nc, in_maps, core_ids=list(range(NCORES)),
                               trace=_trace)
    out = np.concatenate([r["y"] for r in res.results], axis=0)
    if _trace:
        kernel._last_result = res
    return out


# revision 27
# speedup vs baseline: 1.3749x; 1.3749x over previous
"""Trainium2 Bass kernel for 2-layer LSTM (H=64) + linear head. bf16 v3.

Math (PyTorch gate order i,f,g,o):
  per layer: z = W_hh @ h + W_ih @ x + b;  i,f,o = sigmoid; g = tanh
             c = f*c + i*g ; h = o*tanh(c)
  out = h2[:, -1] @ Wlin.T + blin

Per-chunk activation funcs (the two gate chunks use separate ACT instrs,
which also lets ACT-A start while chunk1's matmuls still run):
  chunk0 [i;f] -> Sigmoid (true si, sf)
  chunk1 [g;o] -> Tanh; o rows pre-scaled 0.5 so yo = 2*sigmoid(z_o)-1
Elementwise per step (TT ops get bf16 2x mode; the one STT is 1x):
  u  = si * tg                # TT   = i*g
  v  = sf * C                 # TT   = f*c
  C' = u + v                  # TT   = c'
  TC = tanh(C')               # ACT
  h' = (yo + 1) * TC          # STT  = 2h; consumer weight cols carry 0.5

Kernel layout (per core, batch B_L=256 split into 2 streams of BS=128):
  - State transposed [H, batch], layers fused along free dim (cols 0:BS =
    layer1 at t=k, cols BS:2BS = layer2 at t=k-1; layer2 lags one step).
  - rr tile [71, 2BS] bf16: rows 0:64 h, row 64 ones, rows 65:71 x_t^T.
  - psG [128, 4BS] fp32 = one PSUM bank: cols 0:2BS = chunk [i;f],
    cols 2BS:4BS = chunk [g;o]. 6 matmuls per step fill it; ONE Sigmoid ACT
    reads all 512 cols (mid steps).
  - x passed host-transposed [I, T, B] so the per-step DMA is 6 contiguous
    256B runs instead of a 768-descriptor 2-byte gather.
"""

import numpy as np

H = 64
I = 6
O = 6
NCORES = 8


def _build(nc, tc, BL, BS, T, dt):
    import concourse.bass as bass
    from concourse import mybir

    f32 = mybir.dt.float32
    bf16 = mybir.dt.bfloat16
    AF = mybir.ActivationFunctionType
    OP = mybir.AluOpType
    nstreams = BL // BS

    xt_d = nc.dram_tensor("xt", [I, T, BL], bf16, kind="ExternalInput")
    w1_d = nc.dram_tensor("w1", [71, 256], bf16, kind="ExternalInput")
    w2a_d = nc.dram_tensor("w2a", [64, 256], bf16, kind="ExternalInput")
    w2b_d = nc.dram_tensor("w2b", [65, 256], bf16, kind="ExternalInput")
    wl_d = nc.dram_tensor("wl", [65, O], bf16, kind="ExternalInput")
    y_d = nc.dram_tensor("y", [BL, O], f32, kind="ExternalOutput")

    yT = y_d[:, :].rearrange("b o -> o b")                 # [O, BL]

    import contextlib
    ctx = contextlib.ExitStack()
    wp = ctx.enter_context(tc.tile_pool(name="w", bufs=1))
    rrp = ctx.enter_context(tc.tile_pool(name="rr", bufs=3))
    cp = ctx.enter_context(tc.tile_pool(name="c", bufs=2))
    sp = ctx.enter_context(tc.tile_pool(name="s", bufs=3))
    pp = ctx.enter_context(tc.tile_pool(name="ps", bufs=1, space="PSUM"))
    pfp = ctx.enter_context(tc.tile_pool(name="psf", bufs=1, space="PSUM"))

    # --- weights to SBUF ---
    w1c = []
    w2ac = []
    w2bc = []
    for c in range(2):
        t_ = wp.tile([71, 128], bf16, tag=f"w1c{c}")
        nc.sync.dma_start(out=t_, in_=w1_d[:, c * 128:(c + 1) * 128])
        w1c.append(t_)
        t_ = wp.tile([64, 128], bf16, tag=f"w2a{c}")
        nc.sync.dma_start(out=t_, in_=w2a_d[:, c * 128:(c + 1) * 128])
        w2ac.append(t_)
        t_ = wp.tile([65, 128], bf16, tag=f"w2b{c}")
        nc.sync.dma_start(out=t_, in_=w2b_d[:, c * 128:(c + 1) * 128])
        w2bc.append(t_)
    wl = wp.tile([65, O], bf16, tag="wl")
    nc.sync.dma_start(out=wl, in_=wl_d[:, :])

    A = slice(0, BS)          # layer-1 cols
    Bc = slice(BS, 2 * BS)    # layer-2 cols
    F = slice(0, 2 * BS)

    for s in range(nstreams):
        bs0 = s * BS

        # persistent ring tiles
        rr = [rrp.tile([71, 2 * BS], dt, tag=f"rr{s}", name=f"rr{s}_{j}")
              for j in range(3)]
        cst = [cp.tile([128, 2 * BS], dt, tag=f"c{s}", name=f"c{s}_{j}")
               for j in range(2)]
        for t_ in rr:
            nc.vector.memset(t_[0:64, :], 0.0)
            nc.vector.memset(t_[64:65, :], 1.0)
        for t_ in cst:
            nc.vector.memset(t_[64:128, :], 0.0)

        # x for tick 0
        nc.sync.dma_start(out=rr[0][65:71, 0:BS], in_=xt_d[:, 0, bs0:bs0 + BS])

        for k in range(T + 1):
            do1 = k < T
            do2 = k > 0
            cols = F if (do1 and do2) else (A if do1 else Bc)
            rcur = rr[k % 3]
            rnxt = rr[(k + 1) % 3]
            ccur = cst[k % 2]
            cnxt = cst[(k + 1) % 2]

            psA = pp.tile([128, 2 * BS], f32, tag=f"pA{s}")
            psB = pp.tile([128, 2 * BS], f32, tag=f"pB{s}")
            # chunk0 ([i;f] rows -> psA) matmuls first so ACT-A can start
            # while chunk1's matmuls still run
            if do1:
                nc.tensor.matmul(psA[:, A], w1c[0], rcur[0:71, A],
                                 start=True, stop=True)
            if do2:
                nc.tensor.matmul(psA[:, Bc], w2ac[0], rcur[0:64, Bc],
                                 start=True, stop=False)
                nc.tensor.matmul(psA[:, Bc], w2bc[0], rcur[0:65, A],
                                 start=False, stop=True)
            if do1:
                nc.tensor.matmul(psB[:, A], w1c[1], rcur[0:71, A],
                                 start=True, stop=True)
            if do2:
                nc.tensor.matmul(psB[:, Bc], w2ac[1], rcur[0:64, Bc],
                                 start=True, stop=False)
                nc.tensor.matmul(psB[:, Bc], w2bc[1], rcur[0:65, A],
                                 start=False, stop=True)

            # gate activations, split per chunk (ACT-A overlaps chunk1's
            # matmuls): chunk0 [i;f] -> Sigmoid, chunk1 [g;o] -> Tanh with
            # o-rows pre-scaled 0.5 so yo = 2*sigmoid(z_o)-1.
            Y = sp.tile([128, 4 * BS], dt, tag=f"Y{s}")
            nc.scalar.activation(Y[:, cols], psA[:, cols], AF.Sigmoid)
            if s == 1 and do1 and do2:
                # split s1's ACT-B in half: gives the scalar FIFO a slot to
                # run s0's tanh(c) between the halves (s0's path is the
                # period-setter; s1 has chain slack)
                nc.scalar.activation(Y[:, 2 * BS:3 * BS], psB[:, A], AF.Tanh)
                nc.scalar.activation(Y[:, 3 * BS:4 * BS], psB[:, Bc], AF.Tanh)
            else:
                nc.scalar.activation(
                    Y[:, 2 * BS + cols.start:2 * BS + cols.stop],
                    psB[:, cols], AF.Tanh)

            Si = Y[0:64, 0:2 * BS]
            Sf = Y[64:128, 0:2 * BS]
            Tg = Y[0:64, 2 * BS:4 * BS]
            Yo = Y[64:128, 2 * BS:4 * BS]

            u = sp.tile([64, 2 * BS], dt, tag=f"u{s}")
            v = sp.tile([64, 2 * BS], dt, tag=f"v{s}")
            # v = sf * C   [= f*c]  (needs only ACT-A)
            nc.vector.tensor_tensor(
                out=v[:, cols], in0=Sf[:, cols], in1=ccur[64:128, cols],
                op=OP.mult)
            # u = si * tg  [= i*g]
            nc.vector.tensor_tensor(
                out=u[:, cols], in0=Si[:, cols], in1=Tg[:, cols], op=OP.mult)
            # C' = u + v
            nc.vector.tensor_tensor(
                out=cnxt[64:128, cols], in0=u[:, cols], in1=v[:, cols],
                op=OP.add)
            # TC = tanh(C')
            TC = sp.tile([128, 2 * BS], dt, tag=f"TC{s}")
            nc.scalar.activation(TC[64:128, cols], cnxt[64:128, cols], AF.Tanh)
            # h' = (yo + 1) * TC   [= 2h; the 2x is folded into consumer cols]
            nc.vector.scalar_tensor_tensor(
                out=rnxt[0:64, cols], in0=Yo[:, cols], scalar=1.0,
                in1=TC[64:128, cols], op0=OP.add, op1=OP.mult)

            if k + 1 < T:
                nc.sync.dma_start(out=rnxt[65:71, 0:BS],
                                  in_=xt_d[:, k + 1, bs0:bs0 + BS])

        # final linear: y = [Wlin | blin] @ [h2; 1]
        rfin = rr[(T + 1) % 3]
        psF = pfp.tile([O, BS], f32, tag=f"pF{s}")
        nc.tensor.matmul(psF[:, :], wl, rfin[0:65, Bc], start=True, stop=True)
        oF = sp.tile([O, BS], f32, tag=f"oF{s}")
        nc.vector.tensor_copy(oF[:, :], psF[:, :])
        nc.sync.dma_start(out=yT[:, bs0:bs0 + BS], in_=oF)

    ctx.close()


def build_nc(BL=256, BS=128, T=512, dtype="bfloat16"):
    import concourse.bacc as bacc
    import concourse.tile as tile
    from concourse import mybir

    dt = getattr(mybir.dt, dtype)
    nc = bacc.Bacc(None, target_bir_lowering=False)
    with tile.TileContext(nc) as tc:
        _build(nc, tc, BL, BS, T, dt)
    nc.compile()
    return nc


def prep_weights(Wih0, Whh0, bih0, bhh0, Wih1, Whh1, bih1, bhh1, Wlin, blin):
    """Host-side weight prep. Returns dict of bf16 DRAM tensors."""
    import ml_dtypes
    f = np.float32
    bf = ml_dtypes.bfloat16
    b0 = (bih0 + bhh0).astype(f)
    b1 = (bih1 + bhh1).astype(f)

    def oscale(M):  # scale o-gate rows (192:256) by 0.5
        M = M.copy()
        M[192:256] *= 0.5
        return M

    # h' = 2h convention: consumer cols of h get * 0.5
    w1 = oscale(np.concatenate([Whh0 * 0.5, b0[:, None], Wih0],
                               axis=1).astype(f))
    w2a = oscale((Whh1 * 0.5).astype(f))
    w2b = oscale(np.concatenate([Wih1 * 0.5, b1[:, None]], axis=1).astype(f))
    wlin_aug = np.concatenate([Wlin * 0.5, blin[:, None]], axis=1).astype(f)

    return {
        "w1": np.ascontiguousarray(w1.T).astype(bf),       # [71, 256]
        "w2a": np.ascontiguousarray(w2a.T).astype(bf),     # [64, 256]
        "w2b": np.ascontiguousarray(w2b.T).astype(bf),     # [65, 256]
        "wl": np.ascontiguousarray(wlin_aug.T).astype(bf), # [65, 6]
    }


_NC_CACHE = {}


def kernel(x, Wih0, Whh0, bih0, bhh0, Wih1, Whh1, bih1, bhh1, Wlin, blin,
           _trace=False):
    import ml_dtypes
    from concourse.bass_utils import run_bass_kernel_spmd

    x = np.asarray(x, dtype=np.float32)
    B, T, _ = x.shape
    BL = B // NCORES
    key = (BL, T)
    if key not in _NC_CACHE:
        _NC_CACHE[key] = build_nc(BL=BL, BS=BL // 2, T=T)
    nc = _NC_CACHE[key]

    # host transpose to [I, T, B] so per-step DMA slices are contiguous
    xt = np.ascontiguousarray(np.transpose(x, (2, 1, 0))).astype(
        ml_dtypes.bfloat16)

    w = prep_weights(np.asarray(Wih0), np.asarray(Whh0), np.asarray(bih0),
                     np.asarray(bhh0), np.asarray(Wih1), np.asarray(Whh1),
                     np.asarray(bih1), np.asarray(bhh1), np.asarray(Wlin),
                     np.asarray(blin))

    in_maps = []
    for c in range(NCORES):
        m = {"xt": np.ascontiguousarray(xt[:, :, c * BL:(c + 1) * BL])}
        m.update(w)
        in_maps.append(m)

    res = run_bass_kernel_spmd(nc, in_maps, core_ids=list(range(NCORES)),
                               trace=_trace)
    out = np.concatenate([r["y"] for r in res.results], axis=0)
    if _trace:
        kernel._last_result = res
    return out


# revision 28
# speedup vs baseline: 1.7358x; 1.2624x over previous
"""Trainium2 Bass kernel for 2-layer LSTM (H=64) + linear head. bf16 v3.

Math (PyTorch gate order i,f,g,o):
  per layer: z = W_hh @ h + W_ih @ x + b;  i,f,o = sigmoid; g = tanh
             c = f*c + i*g ; h = o*tanh(c)
  out = h2[:, -1] @ Wlin.T + blin

Per-chunk activation funcs (the two gate chunks use separate ACT instrs,
which also lets ACT-A start while chunk1's matmuls still run):
  chunk0 [i;f] -> Sigmoid (true si, sf)
  chunk1 [g;o] -> Tanh; o rows pre-scaled 0.5 so yo = 2*sigmoid(z_o)-1
Elementwise per step (TT ops get bf16 2x mode; the one STT is 1x):
  u  = si * tg                # TT   = i*g
  v  = sf * C                 # TT   = f*c
  C' = u + v                  # TT   = c'
  TC = tanh(C')               # ACT
  h' = (yo + 1) * TC          # STT  = 2h; consumer weight cols carry 0.5

Kernel layout (per core, batch B_L=256 split into 2 streams of BS=128):
  - State transposed [H, batch], layers fused along free dim (cols 0:BS =
    layer1 at t=k, cols BS:2BS = layer2 at t=k-1; layer2 lags one step).
  - rr tile [71, 2BS] bf16: rows 0:64 h, row 64 ones, rows 65:71 x_t^T.
  - psG [128, 4BS] fp32 = one PSUM bank: cols 0:2BS = chunk [i;f],
    cols 2BS:4BS = chunk [g;o]. 6 matmuls per step fill it; ONE Sigmoid ACT
    reads all 512 cols (mid steps).
  - x passed host-transposed [I, T, B] so the per-step DMA is 6 contiguous
    256B runs instead of a 768-descriptor 2-byte gather.
"""

import numpy as np

H = 64
I = 6
O = 6
NCORES = 8


def _build(nc, tc, BL, BS, T, dt):
    import concourse.bass as bass
    from concourse import mybir

    f32 = mybir.dt.float32
    bf16 = mybir.dt.bfloat16
    AF = mybir.ActivationFunctionType
    OP = mybir.AluOpType
    nstreams = BL // BS

    xt_d = nc.dram_tensor("xt", [I, T, BL], bf16, kind="ExternalInput")
    w1_d = nc.dram_tensor("w1", [71, 256], bf16, kind="ExternalInput")
    w2a_d = nc.dram_tensor("w2a", [64, 256], bf16, kind="ExternalInput")
    w2b_d = nc.dram_tensor("w2b", [65, 256], bf16, kind="ExternalInput")
    wl_d = nc.dram_tensor("wl", [65, O], bf16, kind="ExternalInput")
    y_d = nc.dram_tensor("y", [BL, O], f32, kind="ExternalOutput")

    yT = y_d[:, :].rearrange("b o -> o b")                 # [O, BL]

    import contextlib
    ctx = contextlib.ExitStack()
    wp = ctx.enter_context(tc.tile_pool(name="w", bufs=1))
    rrp = ctx.enter_context(tc.tile_pool(name="rr", bufs=3))
    cp = ctx.enter_context(tc.tile_pool(name="c", bufs=2))
    sp = ctx.enter_context(tc.tile_pool(name="s", bufs=3))
    pp = ctx.enter_context(tc.tile_pool(name="ps", bufs=1, space="PSUM"))
    pfp = ctx.enter_context(tc.tile_pool(name="psf", bufs=1, space="PSUM"))

    # --- weights to SBUF ---
    w1c = []
    w2ac = []
    w2bc = []
    for c in range(2):
        t_ = wp.tile([71, 128], bf16, tag=f"w1c{c}")
        nc.sync.dma_start(out=t_, in_=w1_d[:, c * 128:(c + 1) * 128])
        w1c.append(t_)
        t_ = wp.tile([64, 128], bf16, tag=f"w2a{c}")
        nc.sync.dma_start(out=t_, in_=w2a_d[:, c * 128:(c + 1) * 128])
        w2ac.append(t_)
        t_ = wp.tile([65, 128], bf16, tag=f"w2b{c}")
        nc.sync.dma_start(out=t_, in_=w2b_d[:, c * 128:(c + 1) * 128])
        w2bc.append(t_)
    wl = wp.tile([65, O], bf16, tag="wl")
    nc.sync.dma_start(out=wl, in_=wl_d[:, :])

    A = slice(0, BS)          # layer-1 cols
    Bc = slice(BS, 2 * BS)    # layer-2 cols
    F = slice(0, 2 * BS)

    for s in range(nstreams):
        bs0 = s * BS

        # persistent ring tiles
        rr = [rrp.tile([71, 2 * BS], dt, tag=f"rr{s}", name=f"rr{s}_{j}")
              for j in range(3)]
        cst = [cp.tile([128, 2 * BS], dt, tag=f"c{s}", name=f"c{s}_{j}")
               for j in range(2)]
        for t_ in rr:
            nc.vector.memset(t_[0:64, :], 0.0)
            nc.vector.memset(t_[64:65, :], 1.0)
        for t_ in cst:
            nc.vector.memset(t_[64:128, :], 0.0)

        # x for tick 0
        nc.sync.dma_start(out=rr[0][65:71, 0:BS], in_=xt_d[:, 0, bs0:bs0 + BS])

        for k in range(T + 1):
            do1 = k < T
            do2 = k > 0
            cols = F if (do1 and do2) else (A if do1 else Bc)
            rcur = rr[k % 3]
            rnxt = rr[(k + 1) % 3]
            ccur = cst[k % 2]
            cnxt = cst[(k + 1) % 2]

            psA = pp.tile([128, 2 * BS], f32, tag=f"pA{s}")
            psB = pp.tile([128, 2 * BS], f32, tag=f"pB{s}")
            # chunk0 ([i;f] rows -> psA) matmuls first so ACT-A can start
            # while chunk1's matmuls still run
            if do1:
                nc.tensor.matmul(psA[:, A], w1c[0], rcur[0:71, A],
                                 start=True, stop=True)
            if do2:
                nc.tensor.matmul(psA[:, Bc], w2ac[0], rcur[0:64, Bc],
                                 start=True, stop=False)
                nc.tensor.matmul(psA[:, Bc], w2bc[0], rcur[0:65, A],
                                 start=False, stop=True)
            if do1:
                nc.tensor.matmul(psB[:, A], w1c[1], rcur[0:71, A],
                                 start=True, stop=True)
            if do2:
                nc.tensor.matmul(psB[:, Bc], w2ac[1], rcur[0:64, Bc],
                                 start=True, stop=False)
                nc.tensor.matmul(psB[:, Bc], w2bc[1], rcur[0:65, A],
                                 start=False, stop=True)

            # gate activations, split per chunk (ACT-A overlaps chunk1's
            # matmuls): chunk0 [i;f] -> Sigmoid, chunk1 [g;o] -> Tanh with
            # o-rows pre-scaled 0.5 so yo = 2*sigmoid(z_o)-1.
            Y = sp.tile([128, 4 * BS], dt, tag=f"Y{s}")
            nc.scalar.activation(Y[:, cols], psA[:, cols], AF.Sigmoid)
            nc.scalar.activation(Y[:, 2 * BS + cols.start:2 * BS + cols.stop],
                                 psB[:, cols], AF.Tanh)

            Si = Y[0:64, 0:2 * BS]
            Sf = Y[64:128, 0:2 * BS]
            Tg = Y[0:64, 2 * BS:4 * BS]
            Yo = Y[64:128, 2 * BS:4 * BS]

            u = sp.tile([64, 2 * BS], dt, tag=f"u{s}")
            v = sp.tile([64, 2 * BS], dt, tag=f"v{s}")
            # v = sf * C   [= f*c]  (needs only ACT-A)
            nc.vector.tensor_tensor(
                out=v[:, cols], in0=Sf[:, cols], in1=ccur[64:128, cols],
                op=OP.mult)
            # u = si * tg  [= i*g]
            nc.vector.tensor_tensor(
                out=u[:, cols], in0=Si[:, cols], in1=Tg[:, cols], op=OP.mult)
            # C' = u + v
            nc.vector.tensor_tensor(
                out=cnxt[64:128, cols], in0=u[:, cols], in1=v[:, cols],
                op=OP.add)
            # TC = tanh(C')
            TC = sp.tile([128, 2 * BS], dt, tag=f"TC{s}")
            nc.scalar.activation(TC[64:128, cols], cnxt[64:128, cols], AF.Tanh)
            # h' = (yo + 1) * TC   [= 2h; the 2x is folded into consumer cols]
            nc.vector.scalar_tensor_tensor(
                out=rnxt[0:64, cols], in0=Yo[:, cols], scalar=1.0,
                in1=TC[64:128, cols], op0=OP.add, op1=OP.mult)

            if k + 1 < T:
                nc.sync.dma_start(out=rnxt[65:71, 0:BS],
                                  in_=xt_d[:, k + 1, bs0:bs0 + BS])

        # final linear: y = [Wlin | blin] @ [h2; 1]
        rfin = rr[(T + 1) % 3]
        psF = pfp.tile([O, BS], f32, tag=f"pF{s}")
        nc.tensor.matmul(psF[:, :], wl, rfin[0:65, Bc], start=True, stop=True)
        oF = sp.tile([O, BS], f32, tag=f"oF{s}")
        nc.vector.tensor_copy(oF[:, :], psF[:, :])
        nc.sync.dma_start(out=yT[:, bs0:bs0 + BS], in_=oF)

    ctx.close()


def build_nc(BL=256, BS=128, T=512, dtype="bfloat16"):
    import concourse.bacc as bacc
    import concourse.tile as tile
    from concourse import mybir

    dt = getattr(mybir.dt, dtype)
    nc = bacc.Bacc(None, target_bir_lowering=False)
    with tile.TileContext(nc) as tc:
        _build(nc, tc, BL, BS, T, dt)
    nc.compile()
    return nc


def prep_weights(Wih0, Whh0, bih0, bhh0, Wih1, Whh1, bih1, bhh1, Wlin, blin):
    """Host-side weight prep. Returns dict of bf16 DRAM tensors."""
    import ml_dtypes
    f = np.float32
    bf = ml_dtypes.bfloat16
    b0 = (bih0 + bhh0).astype(f)
    b1 = (bih1 + bhh1).astype(f)

    def oscale(M):  # scale o-gate rows (192:256) by 0.5
        M = M.copy()
        M[192:256] *= 0.5
        return M

    # h' = 2h convention: consumer cols of h get * 0.5
    w1 = oscale(np.concatenate([Whh0 * 0.5, b0[:, None], Wih0],
                               axis=1).astype(f))
    w2a = oscale((Whh1 * 0.5).astype(f))
    w2b = oscale(np.concatenate([Wih1 * 0.5, b1[:, None]], axis=1).astype(f))
    wlin_aug = np.concatenate([Wlin * 0.5, blin[:, None]], axis=1).astype(f)

    return {
        "w1": np.ascontiguousarray(w1.T).astype(bf),       # [71, 256]
        "w2a": np.ascontiguousarray(w2a.T).astype(bf),     # [64, 256]
        "w2b": np.ascontiguousarray(w2b.T).astype(bf),     # [65, 256]
        "wl": np.ascontiguousarray(wlin_aug.T).astype(bf), # [65, 6]
    }


_NC_CACHE = {}


def kernel(x, Wih0, Whh0, bih0, bhh0, Wih1, Whh1, bih1, bhh1, Wlin, blin,
           _trace=False):
    import ml_dtypes
    from concourse.bass_utils import run_bass_kernel_spmd

    x = np.asarray(x, dtype=np.float32)
    B, T, _ = x.shape
    BL = B // NCORES
    key = (BL, T)
    if key not in _NC_CACHE:
        _NC_CACHE[key] = build_nc(BL=BL, BS=BL // 2, T=T)
    nc = _NC_CACHE[key]

    # host transpose to [I, T, B] so per-step DMA slices are contiguous
    xt = np.ascontiguousarray(np.transpose(x, (2, 1, 0))).astype(
        ml_dtypes.bfloat16)

    w = prep_weights(np.asarray(Wih0), np.asarray(Whh0), np.asarray(bih0),
                     np.asarray(bhh0), np.asarray(Wih1), np.asarray(Whh1),
                     np.asarray(bih1), np.asarray(bhh1), np.asarray(Wlin),
                     np.asarray(blin))

    in_maps = []
    for c in range(NCORES):
        m = {"xt": np.ascontiguousarray(xt[:, :, c * BL:(c + 1) * BL])}
        m.update(w)
        in_maps.append(m)

    res = run_bass_kernel_spmd(nc, in_maps, core_ids=list(range(NCORES)),
                               trace=_trace)
    out = np.concatenate([r["y"] for r in res.results], axis=0)
    if _trace:
        kernel._last_result = res
    return out
